# revision 1
# baseline (speedup 1.0000x reference)
"""Trainium2 Bass kernel for CustomAttentionWithPE.

Reference computation (B=2, S=2048, H=16, Dh=64, D=1024):
    qkv = hs @ W_qkv + b_qkv ; split to q,k,v per head
    q,k = RoPE(q), RoPE(k)
    out = softmax(q k^T / 8) v   (no mask)
    return concat_heads(out) @ W_o + b_o

Sharding: 8 cores -> (batch b = core//4, head-quad g = core%4, heads 4g..4g+3).
Each core computes partial = attn(heads of g, batch b) @ W_o[rows of g]
for its batch; host sums the 4 partials per batch and adds the bias terms
(b_o and the V-bias contribution b_v @ W_o; softmax rows sum to 1 so the
V bias contributes exactly b_v @ W_o per token).

Device pipeline per core (all fp32):
  xT [1024, 2048] (= hidden_states[b].T) streamed in 128x512 tiles.
  QT/KT computed transposed ([dh-rows, token-cols]) so RoPE and the
  scores matmul need no transposes: QT = Wq.T @ x.T via PE.
  V computed in natural [token, vcol] layout (PV contraction needs
  token on partitions), augmented with a ones column so the PV matmul
  also produces the softmax denominator Z (row 64 of its PSUM output).
  scores^T[k,q] per head via PE (contract dh=64); two heads of a pair
  occupy disjoint PE row groups (base partitions 0 / 64) so their
  matmuls can overlap via tile_position row packing.
  exp on ScalarE directly PSUM->SBUF in [128, 1024] groups (2 k-tiles),
  scale=0.125 folds in the 1/sqrt(dh) score scaling.
  PV accumulates over the 16 k-tiles into PSUM [65, 512]; normalization
  multiplies by 1/Z broadcast across partitions via a rank-1 PE matmul.
  Output projection contracts the 256 local head-dims in 2 chunks of 128.
"""

import math
from contextlib import ExitStack

import numpy as np

import concourse.bass as bass
import concourse.mybir as mybir
import concourse.tile as tile
from concourse.bass_utils import run_bass_kernel_spmd

F32 = mybir.dt.float32
AF = mybir.ActivationFunctionType

B, S, D = 2, 2048, 1024
NH, HD = 16, 64
ROPE_BASE = 10000.0
N_CORES = 8
HPC = 4  # heads per core
DLOC = HPC * HD  # 256 local head dims per core


def _split_sync_waits(nc, maxw=1):
    """This container's walrus rejects >1-2 SyncWaits per instruction
    ("Too many sync wait commands"). Move excess waits onto NoOps."""
    for f in nc.m.functions:
        for blk in f.blocks:
            new_instructions = []
            for ins in blk.instructions:
                si = getattr(ins, "sync_info", None)
                if si is not None and si.on_wait and len(si.on_wait) > maxw:
                    waits = list(si.on_wait)
                    extra, keep = waits[:-maxw], waits[-maxw:]
                    si.on_wait = keep
                    for i in range(0, len(extra), maxw):
                        nop = mybir.InstNoOp(
                            name=nc.get_next_instruction_name(),
                            engine=ins.engine,
                            sync_info=mybir.SyncInfo(
                                on_wait=extra[i : i + maxw], on_update=[]
                            ),
                        )
                        nc.register_instruction(nop, overwrite=True)
                        new_instructions.append(nop)
                new_instructions.append(ins)
            blk.instructions[:] = new_instructions


def build_attention_nc(seq=S, add_qk_bias=False):
    """One SPMD program; per-core data differs only through inputs."""
    nc = bass.Bass()
    NT = seq // 512  # 512-token stripes
    KT = seq // 128  # k tiles
    QG = 2  # k-tiles per exp group
    NCH = D // 128  # contraction chunks over d_model

    xT = nc.dram_tensor("xT", [D, seq], F32, kind="ExternalInput")
    wq = nc.dram_tensor("wq", [D, DLOC], F32, kind="ExternalInput")
    wk = nc.dram_tensor("wk", [D, DLOC], F32, kind="ExternalInput")
    wv = nc.dram_tensor("wv", [D, DLOC], F32, kind="ExternalInput")
    wo = nc.dram_tensor("wo", [DLOC, D], F32, kind="ExternalInput")
    cosT = nc.dram_tensor("cosT", [HD, seq], F32, kind="ExternalInput")
    sinT = nc.dram_tensor("sinT", [HD, seq], F32, kind="ExternalInput")
    bqk = nc.dram_tensor("bqk", [2, DLOC], F32, kind="ExternalInput")
    out = nc.dram_tensor("out", [seq, D], F32, kind="ExternalOutput")

    with tile.TileContext(nc) as tc, ExitStack() as ctx:
        consts = ctx.enter_context(tc.tile_pool(name="consts", bufs=1))
        # weights as [128, chunk, cols]; row d = c*128 + p
        wq_sb = consts.tile([128, NCH, DLOC], F32)
        nc.sync.dma_start(out=wq_sb, in_=wq.rearrange("(c p) m -> p c m", p=128))
        wk_sb = consts.tile([128, NCH, DLOC], F32)
        nc.sync.dma_start(out=wk_sb, in_=wk.rearrange("(c p) m -> p c m", p=128))
        wv_sb = consts.tile([128, NCH, DLOC], F32)
        nc.sync.dma_start(out=wv_sb, in_=wv.rearrange("(c p) m -> p c m", p=128))
        wo_sb = consts.tile([128, 2, D], F32)
        nc.sync.dma_start(out=wo_sb, in_=wo.rearrange("(c p) m -> p c m", p=128))
        # cos/sin rows duplicated for the two heads of a pair
        cs_sb = consts.tile([128, seq], F32)
        nc.sync.dma_start(out=cs_sb[0:HD, :], in_=cosT[:])
        nc.sync.dma_start(out=cs_sb[HD:128, :], in_=cosT[:])
        sn_sb = consts.tile([128, seq], F32)
        nc.sync.dma_start(out=sn_sb[0:HD, :], in_=sinT[:])
        nc.sync.dma_start(out=sn_sb[HD:128, :], in_=sinT[:])
        ones_sb = consts.tile([128, HD], F32)
        nc.vector.memset(ones_sb, 1.0)
        if add_qk_bias:
            bqk_sb = consts.tile([128, 2, 2], F32)
            nc.sync.dma_start(
                out=bqk_sb, in_=bqk.rearrange("b (h p) -> p b h", p=128)
            )

        # long-lived activation tensors
        acts = ctx.enter_context(tc.tile_pool(name="acts", bufs=1))
        qtr = acts.tile([128, 2, seq], F32)  # RoPE'd Q^T, head pairs
        ktr = acts.tile([128, 2, seq], F32)
        v_sb = acts.tile([128, KT, HPC, HD + 1], F32)  # V natural + ones col
        att = acts.tile([128, 2, seq], F32)  # normalized attn out ^T
        nc.vector.memset(v_sb[:, :, :, HD : HD + 1], 1.0)

        # ---------------- phase 1: QKV projection + RoPE -------------
        with ExitStack() as p1:
            xpool = p1.enter_context(tc.tile_pool(name="xT", bufs=NCH + 2))
            qraw_pool = p1.enter_context(tc.tile_pool(name="qraw", bufs=1))
            ps1 = p1.enter_context(
                tc.tile_pool(name="ps1", bufs=2, space="PSUM")
            )
            rope_tmp = p1.enter_context(tc.tile_pool(name="ropetmp", bufs=2))

            qt_raw = qraw_pool.tile([128, 2, seq], F32)
            kt_raw = qraw_pool.tile([128, 2, seq], F32)

            for nt in range(NT):
                xts = []
                for c in range(NCH):
                    xt = xpool.tile([128, 512], F32, tag="xt")
                    nc.sync.dma_start(
                        out=xt, in_=xT[c * 128 : (c + 1) * 128, nt * 512 : nt * 512 + 512]
                    )
                    xts.append(xt)
                for hp in range(2):
                    for dst, w in ((qt_raw, wq_sb), (kt_raw, wk_sb)):
                        ps = ps1.tile([128, 512], F32, tag="qk")
                        for c in range(NCH):
                            nc.tensor.matmul(
                                ps,
                                w[:, c, hp * 128 : hp * 128 + 128],
                                xts[c],
                                start=(c == 0),
                                stop=(c == NCH - 1),
                            )
                        nc.vector.tensor_copy(
                            dst[:, hp, nt * 512 : nt * 512 + 512], ps
                        )
                # V natural: out [128 tokens, 256 vcols]
                for tt in range(4):
                    ps = ps1.tile([128, DLOC], F32, tag="v")
                    for c in range(NCH):
                        nc.tensor.matmul(
                            ps,
                            xts[c][:, tt * 128 : tt * 128 + 128],
                            wv_sb[:, c, :],
                            start=(c == 0),
                            stop=(c == NCH - 1),
                        )
                    kt_idx = nt * 4 + tt
                    for h in range(HPC):
                        nc.vector.tensor_copy(
                            v_sb[:, kt_idx, h, 0:HD], ps[:, h * HD : (h + 1) * HD]
                        )

            if add_qk_bias:
                for hp in range(2):
                    nc.vector.tensor_scalar_add(
                        qt_raw[:, hp, :], qt_raw[:, hp, :], bqk_sb[:, 0, hp : hp + 1]
                    )
                    nc.vector.tensor_scalar_add(
                        kt_raw[:, hp, :], kt_raw[:, hp, :], bqk_sb[:, 1, hp : hp + 1]
                    )

            # RoPE: dst = raw*cos + rot(raw)*sin ; rot rows (per 64-block):
            # [0:32] = -raw[32:64], [32:64] = +raw[0:32]
            for raw, dst in ((qt_raw, qtr), (kt_raw, ktr)):
                for hp in range(2):
                    for nt in range(NT):
                        cs = slice(nt * 512, nt * 512 + 512)
                        rot = rope_tmp.tile([128, 512], F32, tag="rot")
                        for base in (0, 64):
                            nc.vector.tensor_scalar_mul(
                                rot[base : base + 32, :],
                                raw[base + 32 : base + 64, hp, cs],
                                -1.0,
                            )
                            nc.vector.tensor_copy(
                                rot[base + 32 : base + 64, :],
                                raw[base : base + 32, hp, cs],
                            )
                        tmp = rope_tmp.tile([128, 512], F32, tag="tmp")
                        nc.vector.tensor_mul(tmp, raw[:, hp, cs], cs_sb[:, cs])
                        nc.vector.tensor_mul(rot, rot, sn_sb[:, cs])
                        nc.vector.tensor_add(dst[:, hp, cs], tmp, rot)

        # ---------------- phase 2: attention + output projection -----
        with ExitStack() as p2:
            ps_sc = p2.enter_context(
                tc.tile_pool(name="ps_sc", bufs=2, space="PSUM")
            )
            ps_pv = p2.enter_context(
                tc.tile_pool(name="ps_pv", bufs=2, space="PSUM")
            )
            ps_zb = p2.enter_context(
                tc.tile_pool(name="ps_zb", bufs=1, space="PSUM")
            )
            ps_wo = p2.enter_context(
                tc.tile_pool(name="ps_wo", bufs=1, space="PSUM")
            )
            slab = p2.enter_context(tc.tile_pool(name="slab", bufs=4))
            npool = p2.enter_context(tc.tile_pool(name="norm", bufs=4))
            opool = p2.enter_context(tc.tile_pool(name="ostage", bufs=2))

            for qt in range(NT):
                qs = slice(qt * 512, qt * 512 + 512)
                for hp in range(2):
                    pv = [
                        ps_pv.tile([128, 512], F32, tag="pv", name="pv0"),
                        ps_pv.tile([128, 512], F32, tag="pv", name="pv1"),
                    ]
                    for g in range(KT // QG):
                        sc = [
                            ps_sc.tile([128, QG * 512], F32, tag="sc", name="sc0"),
                            ps_sc.tile([128, QG * 512], F32, tag="sc", name="sc1"),
                        ]
                        for j in range(QG):
                            kt_idx = g * QG + j
                            for h in range(2):
                                hb = h * 64
                                nc.tensor.matmul(
                                    sc[h][:, j * 512 : j * 512 + 512],
                                    ktr[
                                        hb : hb + 64,
                                        hp,
                                        kt_idx * 128 : kt_idx * 128 + 128,
                                    ],
                                    qtr[hb : hb + 64, hp, qs],
                                    start=True,
                                    stop=True,
                                )
                        pt = [
                            slab.tile([128, QG * 512], F32, tag="pt", name="pt0"),
                            slab.tile([128, QG * 512], F32, tag="pt", name="pt1"),
                        ]
                        for h in range(2):
                            nc.scalar.activation(pt[h], sc[h], AF.Exp, scale=0.125)
                        for j in range(QG):
                            kt_idx = g * QG + j
                            for h in range(2):
                                nc.tensor.matmul(
                                    pv[h][0 : HD + 1, :],
                                    v_sb[:, kt_idx, hp * 2 + h, :],
                                    pt[h][:, j * 512 : j * 512 + 512],
                                    start=(kt_idx == 0),
                                    stop=(kt_idx == KT - 1),
                                    skip_group_check=True,
                                )
                    # normalize: att[h-rows, hp, qs] = pv[0:64] * (1/Z bcast)
                    for h in range(2):
                        hb = h * 64
                        o_sb = npool.tile([128, 512], F32, tag="osb")
                        nc.vector.tensor_copy(o_sb[hb : hb + 64, :], pv[h][0:HD, :])
                        zrow = npool.tile([128, 512], F32, tag="z")
                        nc.vector.reciprocal(
                            zrow[HD : HD + 1, :], pv[h][HD : HD + 1, :]
                        )
                        zb = ps_zb.tile([128, 512], F32, tag="zb")
                        nc.tensor.matmul(
                            zb[hb : hb + 64, :],
                            ones_sb[HD : HD + 1, 0:HD],
                            zrow[HD : HD + 1, :],
                            start=True,
                            stop=True,
                        )
                        nc.vector.tensor_mul(
                            att[hb : hb + 64, hp, qs],
                            o_sb[hb : hb + 64, :],
                            zb[hb : hb + 64, :],
                        )
                # output projection for this 512-token stripe
                for tt in range(4):
                    tok = qt * 512 + tt * 128
                    for nh in range(2):
                        ps = ps_wo.tile([128, 512], F32, tag="wo")
                        for hp in range(2):
                            nc.tensor.matmul(
                                ps,
                                att[:, hp, tok : tok + 128],
                                wo_sb[:, hp, nh * 512 : nh * 512 + 512],
                                start=(hp == 0),
                                stop=(hp == 1),
                            )
                        o_out = opool.tile([128, 512], F32, tag="oo")
                        nc.vector.tensor_copy(o_out, ps)
                        nc.sync.dma_start(
                            out=out[tok : tok + 128, nh * 512 : nh * 512 + 512],
                            in_=o_out,
                        )

    _split_sync_waits(nc, maxw=1)
    return nc


_NC_CACHE = {}


def _rope_cos_sin(seq):
    inv_freq = 1.0 / (
        ROPE_BASE ** (np.arange(0, HD, 2, dtype=np.float32) / HD)
    )
    pos = np.arange(seq, dtype=np.float32)
    freqs = pos[:, None] * inv_freq[None, :]  # [seq, 32]
    emb = np.concatenate([freqs, freqs], axis=-1)  # [seq, 64]
    return np.cos(emb).astype(np.float32), np.sin(emb).astype(np.float32)


def kernel(hidden_states, W_qkv, b_qkv, W_o, b_o):
    hs = np.asarray(hidden_states, dtype=np.float32)
    W_qkv = np.asarray(W_qkv, dtype=np.float32)
    b_qkv = np.asarray(b_qkv, dtype=np.float32)
    W_o = np.asarray(W_o, dtype=np.float32)
    b_o = np.asarray(b_o, dtype=np.float32)
    b, seq, d = hs.shape

    bq, bk, bv = b_qkv[:D], b_qkv[D : 2 * D], b_qkv[2 * D :]
    add_qk_bias = bool(np.any(bq) or np.any(bk))

    key = (seq, add_qk_bias)
    if key not in _NC_CACHE:
        _NC_CACHE[key] = build_attention_nc(seq, add_qk_bias)
    nc = _NC_CACHE[key]

    cos, sin = _rope_cos_sin(seq)
    cosT = np.ascontiguousarray(cos.T)
    sinT = np.ascontiguousarray(sin.T)

    in_maps = []
    for core in range(N_CORES):
        bb, g = core // 4, core % 4
        cols = slice(g * DLOC, (g + 1) * DLOC)
        in_maps.append(
            {
                "xT": np.ascontiguousarray(hs[bb].T),
                "wq": np.ascontiguousarray(W_qkv[:, cols]),
                "wk": np.ascontiguousarray(W_qkv[:, 1024:][:, cols]),
                "wv": np.ascontiguousarray(W_qkv[:, 2048:][:, cols]),
                "wo": np.ascontiguousarray(W_o[cols, :]),
                "cosT": cosT,
                "sinT": sinT,
                "bqk": np.stack([bq[cols], bk[cols]]),
            }
        )

    res = run_bass_kernel_spmd(nc, in_maps, list(range(N_CORES)))
    parts = [res.results[c]["out"] for c in range(N_CORES)]
    outv = np.stack(
        [parts[0] + parts[1] + parts[2] + parts[3],
         parts[4] + parts[5] + parts[6] + parts[7]]
    )
    outv += b_o[None, None, :] + (bv @ W_o)[None, None, :]
    return outv.astype(np.float32)



# revision 8
# speedup vs baseline: 2.4903x; 2.4903x over previous
"""Trainium2 Bass kernel for CustomAttentionWithPE (bf16 pipeline).

Reference computation (B=2, S=2048, H=16, Dh=64, D=1024):
    qkv = hs @ W_qkv + b_qkv ; split to q,k,v per head
    q,k = RoPE(q), RoPE(k)
    out = softmax(q k^T / 8) v   (no mask)
    return concat_heads(out) @ W_o + b_o

Sharding: 8 cores -> (batch b = core//4, head-quad g = core%4, heads 4g..4g+3).
Each core computes partial = attn(heads of g, batch b) @ W_o[rows of g]
for its batch; host sums the 4 partials per batch and adds the bias terms
(b_o and b_v @ W_o; softmax rows sum to 1 so the V bias contributes exactly
b_v @ W_o per token).

v2 design (vs fp32 baseline at ~800us):
  - All matmuls in bf16 (1 cyc/row vs fp32's 4); inputs converted to bf16
    on host, halving DMA. PSUM accumulation stays fp32.
  - Softmax exp on ScalarE is the new bottleneck (~147us: 16.8M elems +
    352cyc/instr overhead). Program order starts attention as early as
    possible (K-hp0 + Q-qt0 projections first) and interleaves V/remaining
    QKV/WO under the exp-bound attention phases so ACT is never starved.
  - Score PSUM tiles are [128, 2(heads), 512] so ONE activation instruction
    exps both heads of a pair per k-tile (fewer fixed overheads).
  - 1/Z via reciprocal_approx_fast (f32, ~5x faster than DVE reciprocal)
    directly off the PSUM Z row; broadcast across partitions by a rank-1
    bf16 matmul into the pv bank's upper 64 partitions.
  - RoPE via host-precomputed sign-folded sin (rows 0-31 negated) so the
    rotate-half is 4 shifted tensor_tensor muls + mul + add, all bf16.
"""

import math
from contextlib import ExitStack

import numpy as np
import ml_dtypes

import concourse.bass as bass
import concourse.mybir as mybir
import concourse.tile as tile
from concourse.bass_utils import run_bass_kernel_spmd

F32 = mybir.dt.float32
BF16 = mybir.dt.bfloat16
AF = mybir.ActivationFunctionType
NPBF16 = ml_dtypes.bfloat16

B, S, D = 2, 2048, 1024
NH, HD = 16, 64
ROPE_BASE = 10000.0
N_CORES = 8
HPC = 4  # heads per core
DLOC = HPC * HD  # 256 local head dims per core


def _split_sync_waits(nc, maxw=1):
    """This container's walrus rejects >1-2 SyncWaits per instruction
    ("Too many sync wait commands"). Move excess waits onto NoOps."""
    for f in nc.m.functions:
        for blk in f.blocks:
            new_instructions = []
            for ins in blk.instructions:
                si = getattr(ins, "sync_info", None)
                if si is not None and si.on_wait and len(si.on_wait) > maxw:
                    waits = list(si.on_wait)
                    extra, keep = waits[:-maxw], waits[-maxw:]
                    si.on_wait = keep
                    for i in range(0, len(extra), maxw):
                        nop = mybir.InstNoOp(
                            name=nc.get_next_instruction_name(),
                            engine=ins.engine,
                            sync_info=mybir.SyncInfo(
                                on_wait=extra[i : i + maxw], on_update=[]
                            ),
                        )
                        nc.register_instruction(nop, overwrite=True)
                        new_instructions.append(nop)
                new_instructions.append(ins)
            blk.instructions[:] = new_instructions


def build_attention_nc(seq=S, add_qk_bias=False, order="interleaved"):
    """One SPMD program; per-core data differs only through inputs."""
    nc = bass.Bass()
    NT = seq // 512  # 512-token stripes
    KT = seq // 128  # k tiles
    NCH = D // 128  # contraction chunks over d_model

    xT = nc.dram_tensor("xT", [D, seq], BF16, kind="ExternalInput")
    wq = nc.dram_tensor("wq", [D, DLOC], BF16, kind="ExternalInput")
    wk = nc.dram_tensor("wk", [D, DLOC], BF16, kind="ExternalInput")
    wv = nc.dram_tensor("wv", [D, DLOC], BF16, kind="ExternalInput")
    wo = nc.dram_tensor("wo", [DLOC, D], BF16, kind="ExternalInput")
    cosT = nc.dram_tensor("cosT", [HD, seq], BF16, kind="ExternalInput")
    sinT = nc.dram_tensor("sinT", [HD, seq], BF16, kind="ExternalInput")
    bqk = nc.dram_tensor("bqk", [2, DLOC], F32, kind="ExternalInput")
    out = nc.dram_tensor("out", [seq, D], BF16, kind="ExternalOutput")

    with tile.TileContext(nc) as tc, ExitStack() as ctx:
        consts = ctx.enter_context(tc.tile_pool(name="consts", bufs=1))
        # warm the exp table set before any real activation
        warm = consts.tile([1, 2], F32)
        nc.vector.memset(warm, 0.0)
        nc.scalar.activation(warm, warm, AF.Exp, scale=1.0)

        # weights as [128, chunk, cols]; row d = c*128 + p
        wq_sb = consts.tile([128, NCH, DLOC], BF16)
        nc.sync.dma_start(out=wq_sb, in_=wq.rearrange("(c p) m -> p c m", p=128))
        wk_sb = consts.tile([128, NCH, DLOC], BF16)
        nc.sync.dma_start(out=wk_sb, in_=wk.rearrange("(c p) m -> p c m", p=128))
        wv_sb = consts.tile([128, NCH, DLOC], BF16)
        nc.sync.dma_start(out=wv_sb, in_=wv.rearrange("(c p) m -> p c m", p=128))
        wo_sb = consts.tile([128, 2, D], BF16)
        nc.sync.dma_start(out=wo_sb, in_=wo.rearrange("(c p) m -> p c m", p=128))
        # cos/sign-folded-sin rows duplicated for the two heads of a pair
        cs_sb = consts.tile([128, seq], BF16)
        nc.sync.dma_start(out=cs_sb[0:HD, :], in_=cosT[:])
        nc.sync.dma_start(out=cs_sb[HD:128, :], in_=cosT[:])
        sn_sb = consts.tile([128, seq], BF16)
        nc.sync.dma_start(out=sn_sb[0:HD, :], in_=sinT[:])
        nc.sync.dma_start(out=sn_sb[HD:128, :], in_=sinT[:])
        ones_sb = consts.tile([128, HD], BF16)
        nc.vector.memset(ones_sb, 1.0)
        if add_qk_bias:
            bqk_sb = consts.tile([128, 2, 2], F32)
            nc.sync.dma_start(
                out=bqk_sb, in_=bqk.rearrange("b (h p) -> p b h", p=128)
            )

        # full x resident in SBUF (bf16): one DMA per 128-row chunk
        x_sb = consts.tile([128, NCH, seq], BF16)
        for c in range(NCH):
            nc.sync.dma_start(
                out=x_sb[:, c, :], in_=xT[c * 128 : (c + 1) * 128, :]
            )

        # long-lived activation tensors
        acts = ctx.enter_context(tc.tile_pool(name="acts", bufs=1))
        qtr = acts.tile([128, 2, seq], BF16)  # RoPE'd Q^T, head pairs
        ktr = acts.tile([128, 2, seq], BF16)
        v_sb = acts.tile([128, KT, HPC, HD + 1], BF16)  # V natural + ones col
        att = acts.tile([128, 2, seq], BF16)  # normalized attn out ^T
        nc.vector.memset(v_sb[:, :, :, HD : HD + 1], 1.0)

        # pools
        ps = ctx.enter_context(tc.tile_pool(name="ps", bufs=1, space="PSUM"))
        rope_pool = ctx.enter_context(tc.tile_pool(name="rope", bufs=3))
        pt_pool = ctx.enter_context(tc.tile_pool(name="ptp", bufs=6))
        npool = ctx.enter_context(tc.tile_pool(name="norm", bufs=2))
        opool = ctx.enter_context(tc.tile_pool(name="ostage", bufs=2))

        def emit_qk(dst, w_sb, hp, nt, name):
            """Project one [128 dims, 512 tok] stripe of q^T or k^T and RoPE it."""
            cs = slice(nt * 512, nt * 512 + 512)
            pst = ps.tile([128, 512], F32, tag="qkv", bufs=2, name=f"ps_{name}")
            for c in range(NCH):
                nc.tensor.matmul(
                    pst,
                    w_sb[:, c, hp * 128 : hp * 128 + 128],
                    x_sb[:, c, cs],
                    start=(c == 0),
                    stop=(c == NCH - 1),
                )
            raw = rope_pool.tile([128, 512], BF16, tag="raw", name=f"raw_{name}")
            nc.vector.tensor_copy(raw, pst)
            if add_qk_bias:
                bi = 0 if dst is qtr else 1
                nc.vector.tensor_scalar_add(raw, raw, bqk_sb[:, bi, hp : hp + 1])
            rot = rope_pool.tile([128, 512], BF16, tag="rot", name=f"rot_{name}")
            for b in (0, 64):
                nc.vector.tensor_copy(rot[b : b + 32, :], raw[b + 32 : b + 64, :])
                nc.vector.tensor_copy(rot[b + 32 : b + 64, :], raw[b : b + 32, :])
            nc.vector.tensor_mul(rot, rot, sn_sb[:, cs])
            tmp = rope_pool.tile([128, 512], BF16, tag="tmp", name=f"tmp_{name}")
            nc.vector.tensor_mul(tmp, raw, cs_sb[:, cs])
            nc.vector.tensor_add(dst[:, hp, cs], tmp, rot)

        def emit_v(nt):
            """V natural [token, vcol] for one 512-token stripe, all 4 heads."""
            for tt in range(4):
                vp = ps.tile([128, HPC, HD], F32, tag="qkv", bufs=2, name="ps_v")
                for c in range(NCH):
                    nc.tensor.matmul(
                        vp,
                        x_sb[:, c, nt * 512 + tt * 128 : nt * 512 + tt * 128 + 128],
                        wv_sb[:, c, :],
                        start=(c == 0),
                        stop=(c == NCH - 1),
                    )
                kt_idx = nt * 4 + tt
                nc.vector.tensor_copy(v_sb[:, kt_idx, :, 0:HD], vp)

        def emit_attn(qt, hp):
            """scores -> exp -> PV -> normalize for one (q-stripe, head pair)."""
            qs = slice(qt * 512, qt * 512 + 512)
            pv = [
                ps.tile([128, 512], F32, tag="pv", bufs=2, name="pv0"),
                ps.tile([128, 512], F32, tag="pv", bufs=2, name="pv1"),
            ]
            for kt in range(KT):
                sc = ps.tile([128, 2, 512], F32, tag="sc", bufs=2, name="sc")
                for h in range(2):
                    hb = h * 64
                    nc.tensor.matmul(
                        sc[:, h, :],
                        ktr[hb : hb + 64, hp, kt * 128 : kt * 128 + 128],
                        qtr[hb : hb + 64, hp, qs],
                        start=True,
                        stop=True,
                    )
                pt = pt_pool.tile([128, 2, 512], BF16, tag="pt", bufs=6, name="pt")
                nc.scalar.activation(pt, sc, AF.Exp, scale=0.125)
                for h in range(2):
                    nc.tensor.matmul(
                        pv[h][0 : HD + 1, :],
                        v_sb[:, kt, hp * 2 + h, :],
                        pt[:, h, :],
                        start=(kt == 0),
                        stop=(kt == KT - 1),
                        skip_group_check=True,
                    )
            # normalize: att[h rows, hp, qs] = pv[h][0:64] * bcast(1/Z)
            # (tensor_tensor requires same start partition on all operands,
            #  so everything is staged to matching partition bases; plain
            #  copies are exempt and do the partition shifts)
            o_sb = npool.tile([128, 512], BF16, tag="osb", name="osb")
            zstage = npool.tile([65, 512], F32, tag="zstage", name="zstage")
            zrecf = npool.tile([65, 512], F32, tag="zrecf", name="zrecf")
            zrec = npool.tile([65, 512], BF16, tag="zrec", name="zrec")
            zb = ps.tile([128, 512], F32, tag="sc", bufs=2, name="zb")
            for h in range(2):
                hb = h * 64
                nc.vector.tensor_copy(o_sb[hb : hb + 64, :], pv[h][0:HD, :])
                nc.vector.tensor_copy(zstage[hb : hb + 1, :], pv[h][HD : HD + 1, :])
            # one partition-parallel reciprocal covers both heads' Z rows
            # (rows 1..63 are don't-care lanes; same wall time as [1,512])
            nc.vector.reciprocal(zrecf[0:HD + 1, :], zstage[0:HD + 1, :])
            for h in range(2):
                hb = h * 64
                nc.vector.tensor_copy(zrec[hb : hb + 1, :], zrecf[hb : hb + 1, :])
                # rank-1 matmul broadcast of 1/Z across 64 partitions
                nc.tensor.matmul(
                    zb[hb : hb + 64, :],
                    ones_sb[hb : hb + 1, 0:HD],
                    zrec[hb : hb + 1, :],
                    start=True,
                    stop=True,
                    skip_group_check=True,
                )
                nc.vector.tensor_mul(
                    att[hb : hb + 64, hp, qs],
                    o_sb[hb : hb + 64, :],
                    zb[hb : hb + 64, :],
                )

        def emit_wo(qt):
            """Output projection + store for one 512-token stripe."""
            for tt in range(4):
                tok = qt * 512 + tt * 128
                for nh in range(2):
                    pw = ps.tile([128, 512], F32, tag="qkv", bufs=2, name="ps_wo")
                    for hp in range(2):
                        nc.tensor.matmul(
                            pw,
                            att[:, hp, tok : tok + 128],
                            wo_sb[:, hp, nh * 512 : nh * 512 + 512],
                            start=(hp == 0),
                            stop=(hp == 1),
                        )
                    o_out = opool.tile([128, 512], BF16, tag="oo", name="oo")
                    nc.vector.tensor_copy(o_out, pw)
                    nc.sync.dma_start(
                        out=out[tok : tok + 128, nh * 512 : nh * 512 + 512],
                        in_=o_out,
                    )

        if order == "serial":
            import os
            parts = os.environ.get("KPARTS", "qkv,v,attn,wo").split(",")
            if "qkv" in parts:
                for hp in range(2):
                    for nt in range(NT):
                        emit_qk(ktr, wk_sb, hp, nt, f"k{hp}_{nt}")
                        emit_qk(qtr, wq_sb, hp, nt, f"q{hp}_{nt}")
            else:
                nc.vector.memset(qtr, 0.1)
                nc.vector.memset(ktr, 0.1)
            if "v" in parts:
                for nt in range(NT):
                    emit_v(nt)
            else:
                nc.vector.memset(v_sb, 0.1)
            if "attn" in parts:
                for hp in range(2):
                    for qt in range(NT):
                        emit_attn(qt, hp)
            else:
                nc.vector.memset(att, 0.1)
            if "wo" in parts:
                for qt in range(NT):
                    emit_wo(qt)
        else:
            # ---- emission order: get ACT (exp) busy as early as possible ----
            # A: K-hp0 for all stripes, then Q-hp0 for qt0 only.
            for nt in range(NT):
                emit_qk(ktr, wk_sb, 0, nt, f"k0_{nt}")
            emit_qk(qtr, wq_sb, 0, 0, "q0_0")
            # B: attention(hp0) with V stripes + remaining projections
            # interleaved.
            for nt in range(NT):
                emit_v(nt)
            emit_attn(0, 0)
            emit_qk(qtr, wq_sb, 0, 1, "q0_1")
            emit_qk(ktr, wk_sb, 1, 0, "k1_0")
            emit_qk(ktr, wk_sb, 1, 1, "k1_1")
            emit_attn(1, 0)
            emit_qk(ktr, wk_sb, 1, 2, "k1_2")
            emit_qk(ktr, wk_sb, 1, 3, "k1_3")
            emit_qk(qtr, wq_sb, 0, 2, "q0_2")
            emit_attn(2, 0)
            emit_qk(qtr, wq_sb, 0, 3, "q0_3")
            emit_qk(qtr, wq_sb, 1, 0, "q1_0")
            emit_qk(qtr, wq_sb, 1, 1, "q1_1")
            emit_attn(3, 0)
            emit_qk(qtr, wq_sb, 1, 2, "q1_2")
            emit_qk(qtr, wq_sb, 1, 3, "q1_3")
            # C: attention(hp1) + output projection per stripe.
            emit_attn(0, 1)
            emit_wo(0)
            emit_attn(1, 1)
            emit_wo(1)
            emit_attn(2, 1)
            emit_wo(2)
            emit_attn(3, 1)
            emit_wo(3)

    _split_sync_waits(nc, maxw=1)
    return nc


_NC_CACHE = {}


def _rope_cos_sin(seq):
    inv_freq = 1.0 / (
        ROPE_BASE ** (np.arange(0, HD, 2, dtype=np.float32) / HD)
    )
    pos = np.arange(seq, dtype=np.float32)
    freqs = pos[:, None] * inv_freq[None, :]  # [seq, 32]
    emb = np.concatenate([freqs, freqs], axis=-1)  # [seq, 64]
    return np.cos(emb).astype(np.float32), np.sin(emb).astype(np.float32)


def build_in_maps(hs, W_qkv, b_qkv, W_o, b_o):
    """Per-core input dict list (shared by kernel() and test harness)."""
    seq = hs.shape[1]
    cos, sin = _rope_cos_sin(seq)
    # sign-folded sin: rows 0..31 negated (multiplies the shifted-up half)
    sin_hat = sin.copy()
    sin_hat[:, :32] *= -1.0
    cosT = np.ascontiguousarray(cos.T).astype(NPBF16)
    sinT = np.ascontiguousarray(sin_hat.T).astype(NPBF16)

    bq, bk = b_qkv[:D], b_qkv[D : 2 * D]
    in_maps = []
    for core in range(N_CORES):
        bb, g = core // 4, core % 4
        cols = slice(g * DLOC, (g + 1) * DLOC)
        in_maps.append(
            {
                "xT": np.ascontiguousarray(hs[bb].T).astype(NPBF16),
                "wq": np.ascontiguousarray(W_qkv[:, :D][:, cols]).astype(NPBF16),
                "wk": np.ascontiguousarray(
                    W_qkv[:, D : 2 * D][:, cols]
                ).astype(NPBF16),
                "wv": np.ascontiguousarray(W_qkv[:, 2 * D :][:, cols]).astype(
                    NPBF16
                ),
                "wo": np.ascontiguousarray(W_o[cols, :]).astype(NPBF16),
                "cosT": cosT,
                "sinT": sinT,
                "bqk": np.stack([bq[cols], bk[cols]]).astype(np.float32),
            }
        )
    return in_maps


def kernel(hidden_states, W_qkv, b_qkv, W_o, b_o):
    hs = np.asarray(hidden_states, dtype=np.float32)
    W_qkv = np.asarray(W_qkv, dtype=np.float32)
    b_qkv = np.asarray(b_qkv, dtype=np.float32)
    W_o = np.asarray(W_o, dtype=np.float32)
    b_o = np.asarray(b_o, dtype=np.float32)
    b, seq, d = hs.shape

    bq, bk, bv = b_qkv[:D], b_qkv[D : 2 * D], b_qkv[2 * D :]
    add_qk_bias = bool(np.any(bq) or np.any(bk))

    key = (seq, add_qk_bias)
    if key not in _NC_CACHE:
        _NC_CACHE[key] = build_attention_nc(seq, add_qk_bias)
    nc = _NC_CACHE[key]

    in_maps = build_in_maps(hs, W_qkv, b_qkv, W_o, b_o)
    res = run_bass_kernel_spmd(nc, in_maps, list(range(N_CORES)))
    parts = [res.results[c]["out"].astype(np.float32) for c in range(N_CORES)]
    outv = np.stack(
        [parts[0] + parts[1] + parts[2] + parts[3],
         parts[4] + parts[5] + parts[6] + parts[7]]
    )
    outv += b_o[None, None, :] + (bv @ W_o)[None, None, :]
    return outv.astype(np.float32)


# revision 11
# speedup vs baseline: 2.6646x; 1.0700x over previous
"""Trainium2 Bass kernel for CustomAttentionWithPE (bf16 pipeline).

Reference computation (B=2, S=2048, H=16, Dh=64, D=1024):
    qkv = hs @ W_qkv + b_qkv ; split to q,k,v per head
    q,k = RoPE(q), RoPE(k)
    out = softmax(q k^T / 8) v   (no mask)
    return concat_heads(out) @ W_o + b_o

Sharding: 8 cores -> (batch b = core//4, head-quad g = core%4, heads 4g..4g+3).
Each core computes partial = attn(heads of g, batch b) @ W_o[rows of g]
for its batch; host sums the 4 partials per batch and adds the bias terms
(b_o and b_v @ W_o; softmax rows sum to 1 so the V bias contributes exactly
b_v @ W_o per token).

v2 design (vs fp32 baseline at ~800us):
  - All matmuls in bf16 (1 cyc/row vs fp32's 4); inputs converted to bf16
    on host, halving DMA. PSUM accumulation stays fp32.
  - Softmax exp on ScalarE is the new bottleneck (~147us: 16.8M elems +
    352cyc/instr overhead). Program order starts attention as early as
    possible (K-hp0 + Q-qt0 projections first) and interleaves V/remaining
    QKV/WO under the exp-bound attention phases so ACT is never starved.
  - Score PSUM tiles are [128, 2(heads), 512] so ONE activation instruction
    exps both heads of a pair per k-tile (fewer fixed overheads).
  - 1/Z via reciprocal_approx_fast (f32, ~5x faster than DVE reciprocal)
    directly off the PSUM Z row; broadcast across partitions by a rank-1
    bf16 matmul into the pv bank's upper 64 partitions.
  - RoPE via host-precomputed sign-folded sin (rows 0-31 negated) so the
    rotate-half is 4 shifted tensor_tensor muls + mul + add, all bf16.
"""

import math
from contextlib import ExitStack

import numpy as np
import ml_dtypes

import concourse.bass as bass
import concourse.mybir as mybir
import concourse.tile as tile
from concourse.bass_utils import run_bass_kernel_spmd

F32 = mybir.dt.float32
BF16 = mybir.dt.bfloat16
AF = mybir.ActivationFunctionType
NPBF16 = ml_dtypes.bfloat16

B, S, D = 2, 2048, 1024
NH, HD = 16, 64
ROPE_BASE = 10000.0
N_CORES = 8
HPC = 4  # heads per core
DLOC = HPC * HD  # 256 local head dims per core


def _split_sync_waits(nc, maxw=1):
    """This container's walrus rejects >1-2 SyncWaits per instruction
    ("Too many sync wait commands"). Move excess waits onto NoOps."""
    for f in nc.m.functions:
        for blk in f.blocks:
            new_instructions = []
            for ins in blk.instructions:
                si = getattr(ins, "sync_info", None)
                if si is not None and si.on_wait and len(si.on_wait) > maxw:
                    waits = list(si.on_wait)
                    extra, keep = waits[:-maxw], waits[-maxw:]
                    si.on_wait = keep
                    for i in range(0, len(extra), maxw):
                        nop = mybir.InstNoOp(
                            name=nc.get_next_instruction_name(),
                            engine=ins.engine,
                            sync_info=mybir.SyncInfo(
                                on_wait=extra[i : i + maxw], on_update=[]
                            ),
                        )
                        nc.register_instruction(nop, overwrite=True)
                        new_instructions.append(nop)
                new_instructions.append(ins)
            blk.instructions[:] = new_instructions


def build_attention_nc(seq=S, add_qk_bias=False, order="interleaved"):
    """One SPMD program; per-core data differs only through inputs."""
    nc = bass.Bass()
    NT = seq // 512  # 512-token stripes
    KT = seq // 128  # k tiles
    NCH = D // 128  # contraction chunks over d_model

    xT = nc.dram_tensor("xT", [D, seq], BF16, kind="ExternalInput")
    wq = nc.dram_tensor("wq", [D, DLOC], BF16, kind="ExternalInput")
    wk = nc.dram_tensor("wk", [D, DLOC], BF16, kind="ExternalInput")
    wv = nc.dram_tensor("wv", [D, DLOC], BF16, kind="ExternalInput")
    wo = nc.dram_tensor("wo", [DLOC, D], BF16, kind="ExternalInput")
    cosT = nc.dram_tensor("cosT", [HD, seq], BF16, kind="ExternalInput")
    sinT = nc.dram_tensor("sinT", [HD, seq], BF16, kind="ExternalInput")
    bqk = nc.dram_tensor("bqk", [2, DLOC], F32, kind="ExternalInput")
    out = nc.dram_tensor("out", [seq, D], BF16, kind="ExternalOutput")

    with tile.TileContext(nc) as tc, ExitStack() as ctx:
        consts = ctx.enter_context(tc.tile_pool(name="consts", bufs=1))
        # warm the exp table set before any real activation
        warm = consts.tile([1, 2], F32)
        nc.vector.memset(warm, 0.0)
        nc.scalar.activation(warm, warm, AF.Exp, scale=1.0)

        # weights as [128, chunk, cols]; row d = c*128 + p
        wq_sb = consts.tile([128, NCH, DLOC], BF16)
        nc.sync.dma_start(out=wq_sb, in_=wq.rearrange("(c p) m -> p c m", p=128))
        wk_sb = consts.tile([128, NCH, DLOC], BF16)
        nc.sync.dma_start(out=wk_sb, in_=wk.rearrange("(c p) m -> p c m", p=128))
        wv_sb = consts.tile([128, NCH, DLOC], BF16)
        nc.sync.dma_start(out=wv_sb, in_=wv.rearrange("(c p) m -> p c m", p=128))
        wo_sb = consts.tile([128, 2, D], BF16)
        nc.sync.dma_start(out=wo_sb, in_=wo.rearrange("(c p) m -> p c m", p=128))
        # cos/sign-folded-sin rows duplicated for the two heads of a pair
        cs_sb = consts.tile([128, seq], BF16)
        nc.sync.dma_start(out=cs_sb[0:HD, :], in_=cosT[:])
        nc.sync.dma_start(out=cs_sb[HD:128, :], in_=cosT[:])
        sn_sb = consts.tile([128, seq], BF16)
        nc.sync.dma_start(out=sn_sb[0:HD, :], in_=sinT[:])
        nc.sync.dma_start(out=sn_sb[HD:128, :], in_=sinT[:])
        ones_sb = consts.tile([128, HD], BF16)
        nc.vector.memset(ones_sb, 1.0)
        if add_qk_bias:
            bqk_sb = consts.tile([128, 2, 2], F32)
            nc.sync.dma_start(
                out=bqk_sb, in_=bqk.rearrange("b (h p) -> p b h", p=128)
            )

        # full x resident in SBUF (bf16), stripe-major DMAs so the first
        # K-projection matmuls can start after ~1/4 of the transfer
        x_sb = consts.tile([128, NCH, seq], BF16)
        for nt in range(NT):
            for c in range(NCH):
                nc.sync.dma_start(
                    out=x_sb[:, c, nt * 512 : nt * 512 + 512],
                    in_=xT[c * 128 : (c + 1) * 128, nt * 512 : nt * 512 + 512],
                )

        # long-lived activation tensors
        acts = ctx.enter_context(tc.tile_pool(name="acts", bufs=1))
        qtr = acts.tile([128, 2, seq], BF16)  # RoPE'd Q^T, head pairs
        ktr = acts.tile([128, 2, seq], BF16)
        v_sb = acts.tile([128, KT, HPC, HD + 1], BF16)  # V natural + ones col
        att = acts.tile([128, 2, seq], BF16)  # normalized attn out ^T
        nc.vector.memset(v_sb[:, :, :, HD : HD + 1], 1.0)

        # pools
        ps = ctx.enter_context(tc.tile_pool(name="ps", bufs=1, space="PSUM"))
        rope_pool = ctx.enter_context(tc.tile_pool(name="rope", bufs=3))
        pt_pool = ctx.enter_context(tc.tile_pool(name="ptp", bufs=6))
        npool = ctx.enter_context(tc.tile_pool(name="norm", bufs=2))
        opool = ctx.enter_context(tc.tile_pool(name="ostage", bufs=2))

        def emit_qk(dst, w_sb, hp, nt, name):
            """Project one [128 dims, 512 tok] stripe of q^T or k^T and RoPE it."""
            cs = slice(nt * 512, nt * 512 + 512)
            pst = ps.tile([128, 512], F32, tag="qkv", bufs=2, name=f"ps_{name}")
            for c in range(NCH):
                nc.tensor.matmul(
                    pst,
                    w_sb[:, c, hp * 128 : hp * 128 + 128],
                    x_sb[:, c, cs],
                    start=(c == 0),
                    stop=(c == NCH - 1),
                )
            raw = rope_pool.tile([128, 512], BF16, tag="raw", name=f"raw_{name}")
            nc.vector.tensor_copy(raw, pst)
            if add_qk_bias:
                bi = 0 if dst is qtr else 1
                nc.vector.tensor_scalar_add(raw, raw, bqk_sb[:, bi, hp : hp + 1])
            rot = rope_pool.tile([128, 512], BF16, tag="rot", name=f"rot_{name}")
            for b in (0, 64):
                nc.vector.tensor_copy(rot[b : b + 32, :], raw[b + 32 : b + 64, :])
                nc.vector.tensor_copy(rot[b + 32 : b + 64, :], raw[b : b + 32, :])
            nc.vector.tensor_mul(rot, rot, sn_sb[:, cs])
            tmp = rope_pool.tile([128, 512], BF16, tag="tmp", name=f"tmp_{name}")
            nc.vector.tensor_mul(tmp, raw, cs_sb[:, cs])
            nc.vector.tensor_add(dst[:, hp, cs], tmp, rot)

        def emit_v(nt):
            """V natural [token, vcol] for one 512-token stripe, all 4 heads."""
            for tt in range(4):
                vp = ps.tile([128, HPC, HD], F32, tag="qkv", bufs=2, name="ps_v")
                for c in range(NCH):
                    nc.tensor.matmul(
                        vp,
                        x_sb[:, c, nt * 512 + tt * 128 : nt * 512 + tt * 128 + 128],
                        wv_sb[:, c, :],
                        start=(c == 0),
                        stop=(c == NCH - 1),
                    )
                kt_idx = nt * 4 + tt
                nc.vector.tensor_copy(v_sb[:, kt_idx, :, 0:HD], vp)

        def emit_attn(qt, hp, pre_kt=None):
            """scores -> exp -> PV -> normalize for one (q-stripe, head pair).

            pre_kt: optional {kt: callable} emitted before that kt's chunk —
            used to interleave prerequisite/filler work (e.g. V stripes) at
            the right trace position (Tile trace order IS program order)."""
            qs = slice(qt * 512, qt * 512 + 512)
            pv = [
                ps.tile([128, 512], F32, tag="pv", bufs=2, name="pv0"),
                ps.tile([128, 512], F32, tag="pv", bufs=2, name="pv1"),
            ]
            for kt in range(KT):
                if pre_kt and kt in pre_kt:
                    pre_kt[kt]()
                sc = ps.tile([128, 2, 512], F32, tag="sc", bufs=2, name="sc")
                for h in range(2):
                    hb = h * 64
                    nc.tensor.matmul(
                        sc[:, h, :],
                        ktr[hb : hb + 64, hp, kt * 128 : kt * 128 + 128],
                        qtr[hb : hb + 64, hp, qs],
                        start=True,
                        stop=True,
                    )
                pt = pt_pool.tile([128, 2, 512], BF16, tag="pt", bufs=6, name="pt")
                nc.scalar.activation(pt, sc, AF.Exp, scale=0.125)
                for h in range(2):
                    nc.tensor.matmul(
                        pv[h][0 : HD + 1, :],
                        v_sb[:, kt, hp * 2 + h, :],
                        pt[:, h, :],
                        start=(kt == 0),
                        stop=(kt == KT - 1),
                        skip_group_check=True,
                    )
            # normalize: att[h rows, hp, qs] = pv[h][0:64] * bcast(1/Z)
            # (tensor_tensor requires same start partition on all operands,
            #  so everything is staged to matching partition bases; plain
            #  copies are exempt and do the partition shifts)
            o_sb = npool.tile([128, 512], BF16, tag="osb", name="osb")
            zstage = npool.tile([65, 512], F32, tag="zstage", name="zstage")
            zrecf = npool.tile([65, 512], F32, tag="zrecf", name="zrecf")
            zrec = npool.tile([65, 512], BF16, tag="zrec", name="zrec")
            zb = ps.tile([128, 512], F32, tag="qkv", bufs=2, name="zb")
            for h in range(2):
                hb = h * 64
                nc.vector.tensor_copy(o_sb[hb : hb + 64, :], pv[h][0:HD, :])
                nc.vector.tensor_copy(zstage[hb : hb + 1, :], pv[h][HD : HD + 1, :])
            # one partition-parallel reciprocal covers both heads' Z rows
            # (rows 1..63 are don't-care lanes; same wall time as [1,512])
            nc.vector.reciprocal(zrecf[0:HD + 1, :], zstage[0:HD + 1, :])
            for h in range(2):
                hb = h * 64
                nc.vector.tensor_copy(zrec[hb : hb + 1, :], zrecf[hb : hb + 1, :])
                # rank-1 matmul broadcast of 1/Z across 64 partitions
                nc.tensor.matmul(
                    zb[hb : hb + 64, :],
                    ones_sb[hb : hb + 1, 0:HD],
                    zrec[hb : hb + 1, :],
                    start=True,
                    stop=True,
                    skip_group_check=True,
                )
                nc.vector.tensor_mul(
                    att[hb : hb + 64, hp, qs],
                    o_sb[hb : hb + 64, :],
                    zb[hb : hb + 64, :],
                )

        def emit_wo(qt):
            """Output projection + store for one 512-token stripe."""
            for tt in range(4):
                tok = qt * 512 + tt * 128
                for nh in range(2):
                    pw = ps.tile([128, 512], F32, tag="qkv", bufs=2, name="ps_wo")
                    for hp in range(2):
                        nc.tensor.matmul(
                            pw,
                            att[:, hp, tok : tok + 128],
                            wo_sb[:, hp, nh * 512 : nh * 512 + 512],
                            start=(hp == 0),
                            stop=(hp == 1),
                        )
                    o_out = opool.tile([128, 512], BF16, tag="oo", name="oo")
                    nc.vector.tensor_copy(o_out, pw)
                    nc.sync.dma_start(
                        out=out[tok : tok + 128, nh * 512 : nh * 512 + 512],
                        in_=o_out,
                    )

        if order == "serial":
            for hp in range(2):
                for nt in range(NT):
                    emit_qk(ktr, wk_sb, hp, nt, f"k{hp}_{nt}")
                    emit_qk(qtr, wq_sb, hp, nt, f"q{hp}_{nt}")
            for nt in range(NT):
                emit_v(nt)
            for hp in range(2):
                for qt in range(NT):
                    emit_attn(qt, hp)
            for qt in range(NT):
                emit_wo(qt)
        else:
            # Emission (trace) order IS program order — every tensor must be
            # written before it is read. Within that constraint, order also
            # sets scheduler priority: attention (which feeds the bottleneck
            # ScalarE exp stream) is emitted as early as possible, with the
            # remaining projections spread between attns as PE filler.
            for nt in range(NT):
                emit_qk(ktr, wk_sb, 0, nt, f"k0_{nt}")
            emit_qk(qtr, wq_sb, 0, 0, "q0_0")
            # V stripe nt feeds PV k-tiles 4nt..4nt+3 of every attention;
            # interleave them into the first attention's chunks.
            emit_attn(0, 0, pre_kt={0: lambda: emit_v(0),
                                    4: lambda: emit_v(1),
                                    8: lambda: emit_v(2),
                                    12: lambda: emit_v(3)})
            emit_qk(qtr, wq_sb, 0, 1, "q0_1")
            emit_attn(1, 0, pre_kt={4: lambda: emit_qk(qtr, wq_sb, 0, 2, "q0_2"),
                                    8: lambda: emit_qk(qtr, wq_sb, 0, 3, "q0_3"),
                                    12: lambda: emit_qk(ktr, wk_sb, 1, 0, "k1_0")})
            emit_attn(2, 0, pre_kt={0: lambda: emit_qk(ktr, wk_sb, 1, 1, "k1_1"),
                                    6: lambda: emit_qk(ktr, wk_sb, 1, 2, "k1_2"),
                                    12: lambda: emit_qk(ktr, wk_sb, 1, 3, "k1_3")})
            emit_attn(3, 0, pre_kt={0: lambda: emit_qk(qtr, wq_sb, 1, 0, "q1_0"),
                                    6: lambda: emit_qk(qtr, wq_sb, 1, 1, "q1_1"),
                                    12: lambda: emit_qk(qtr, wq_sb, 1, 2, "q1_2")})
            emit_attn(0, 1, pre_kt={0: lambda: emit_qk(qtr, wq_sb, 1, 3, "q1_3")})
            emit_attn(1, 1, pre_kt={4: lambda: emit_wo(0)})
            emit_attn(2, 1, pre_kt={4: lambda: emit_wo(1)})
            emit_attn(3, 1, pre_kt={4: lambda: emit_wo(2)})
            emit_wo(3)

    _split_sync_waits(nc, maxw=1)
    return nc


_NC_CACHE = {}


def _rope_cos_sin(seq):
    inv_freq = 1.0 / (
        ROPE_BASE ** (np.arange(0, HD, 2, dtype=np.float32) / HD)
    )
    pos = np.arange(seq, dtype=np.float32)
    freqs = pos[:, None] * inv_freq[None, :]  # [seq, 32]
    emb = np.concatenate([freqs, freqs], axis=-1)  # [seq, 64]
    return np.cos(emb).astype(np.float32), np.sin(emb).astype(np.float32)


def build_in_maps(hs, W_qkv, b_qkv, W_o, b_o):
    """Per-core input dict list (shared by kernel() and test harness)."""
    seq = hs.shape[1]
    cos, sin = _rope_cos_sin(seq)
    # sign-folded sin: rows 0..31 negated (multiplies the shifted-up half)
    sin_hat = sin.copy()
    sin_hat[:, :32] *= -1.0
    cosT = np.ascontiguousarray(cos.T).astype(NPBF16)
    sinT = np.ascontiguousarray(sin_hat.T).astype(NPBF16)

    bq, bk = b_qkv[:D], b_qkv[D : 2 * D]
    in_maps = []
    for core in range(N_CORES):
        bb, g = core // 4, core % 4
        cols = slice(g * DLOC, (g + 1) * DLOC)
        in_maps.append(
            {
                "xT": np.ascontiguousarray(hs[bb].T).astype(NPBF16),
                "wq": np.ascontiguousarray(W_qkv[:, :D][:, cols]).astype(NPBF16),
                "wk": np.ascontiguousarray(
                    W_qkv[:, D : 2 * D][:, cols]
                ).astype(NPBF16),
                "wv": np.ascontiguousarray(W_qkv[:, 2 * D :][:, cols]).astype(
                    NPBF16
                ),
                "wo": np.ascontiguousarray(W_o[cols, :]).astype(NPBF16),
                "cosT": cosT,
                "sinT": sinT,
                "bqk": np.stack([bq[cols], bk[cols]]).astype(np.float32),
            }
        )
    return in_maps


def kernel(hidden_states, W_qkv, b_qkv, W_o, b_o):
    hs = np.asarray(hidden_states, dtype=np.float32)
    W_qkv = np.asarray(W_qkv, dtype=np.float32)
    b_qkv = np.asarray(b_qkv, dtype=np.float32)
    W_o = np.asarray(W_o, dtype=np.float32)
    b_o = np.asarray(b_o, dtype=np.float32)
    b, seq, d = hs.shape

    bq, bk, bv = b_qkv[:D], b_qkv[D : 2 * D], b_qkv[2 * D :]
    add_qk_bias = bool(np.any(bq) or np.any(bk))

    key = (seq, add_qk_bias)
    if key not in _NC_CACHE:
        _NC_CACHE[key] = build_attention_nc(seq, add_qk_bias)
    nc = _NC_CACHE[key]

    in_maps = build_in_maps(hs, W_qkv, b_qkv, W_o, b_o)
    res = run_bass_kernel_spmd(nc, in_maps, list(range(N_CORES)))
    parts = [res.results[c]["out"].astype(np.float32) for c in range(N_CORES)]
    outv = np.stack(
        [parts[0] + parts[1] + parts[2] + parts[3],
         parts[4] + parts[5] + parts[6] + parts[7]]
    )
    outv += b_o[None, None, :] + (bv @ W_o)[None, None, :]
    return outv.astype(np.float32)


# revision 12
# speedup vs baseline: 2.8247x; 1.0601x over previous
"""Trainium2 Bass kernel for CustomAttentionWithPE (bf16 pipeline).

Reference computation (B=2, S=2048, H=16, Dh=64, D=1024):
    qkv = hs @ W_qkv + b_qkv ; split to q,k,v per head
    q,k = RoPE(q), RoPE(k)
    out = softmax(q k^T / 8) v   (no mask)
    return concat_heads(out) @ W_o + b_o

Sharding: 8 cores -> (batch b = core//4, head-quad g = core%4, heads 4g..4g+3).
Each core computes partial = attn(heads of g, batch b) @ W_o[rows of g]
for its batch; host sums the 4 partials per batch and adds the bias terms
(b_o and b_v @ W_o; softmax rows sum to 1 so the V bias contributes exactly
b_v @ W_o per token).

v2 design (vs fp32 baseline at ~800us):
  - All matmuls in bf16 (1 cyc/row vs fp32's 4); inputs converted to bf16
    on host, halving DMA. PSUM accumulation stays fp32.
  - Softmax exp on ScalarE is the new bottleneck (~147us: 16.8M elems +
    352cyc/instr overhead). Program order starts attention as early as
    possible (K-hp0 + Q-qt0 projections first) and interleaves V/remaining
    QKV/WO under the exp-bound attention phases so ACT is never starved.
  - Score PSUM tiles are [128, 2(heads), 512] so ONE activation instruction
    exps both heads of a pair per k-tile (fewer fixed overheads).
  - 1/Z via reciprocal_approx_fast (f32, ~5x faster than DVE reciprocal)
    directly off the PSUM Z row; broadcast across partitions by a rank-1
    bf16 matmul into the pv bank's upper 64 partitions.
  - RoPE via host-precomputed sign-folded sin (rows 0-31 negated) so the
    rotate-half is 4 shifted tensor_tensor muls + mul + add, all bf16.
"""

import math
from contextlib import ExitStack

import numpy as np
import ml_dtypes

import concourse.bass as bass
import concourse.mybir as mybir
import concourse.tile as tile
from concourse.bass_utils import run_bass_kernel_spmd

F32 = mybir.dt.float32
BF16 = mybir.dt.bfloat16
AF = mybir.ActivationFunctionType
NPBF16 = ml_dtypes.bfloat16

B, S, D = 2, 2048, 1024
NH, HD = 16, 64
ROPE_BASE = 10000.0
N_CORES = 8
HPC = 4  # heads per core
DLOC = HPC * HD  # 256 local head dims per core


def _split_sync_waits(nc, maxw=1):
    """This container's walrus rejects >1-2 SyncWaits per instruction
    ("Too many sync wait commands"). Move excess waits onto NoOps."""
    for f in nc.m.functions:
        for blk in f.blocks:
            new_instructions = []
            for ins in blk.instructions:
                si = getattr(ins, "sync_info", None)
                if si is not None and si.on_wait and len(si.on_wait) > maxw:
                    waits = list(si.on_wait)
                    extra, keep = waits[:-maxw], waits[-maxw:]
                    si.on_wait = keep
                    for i in range(0, len(extra), maxw):
                        nop = mybir.InstNoOp(
                            name=nc.get_next_instruction_name(),
                            engine=ins.engine,
                            sync_info=mybir.SyncInfo(
                                on_wait=extra[i : i + maxw], on_update=[]
                            ),
                        )
                        nc.register_instruction(nop, overwrite=True)
                        new_instructions.append(nop)
                new_instructions.append(ins)
            blk.instructions[:] = new_instructions


def build_attention_nc(seq=S, add_qk_bias=False, order="interleaved"):
    """One SPMD program; per-core data differs only through inputs."""
    nc = bass.Bass()
    NT = seq // 512  # 512-token stripes
    KT = seq // 128  # k tiles
    NCH = D // 128  # contraction chunks over d_model

    xT = nc.dram_tensor("xT", [D, seq], BF16, kind="ExternalInput")
    wq = nc.dram_tensor("wq", [D, DLOC], BF16, kind="ExternalInput")
    wk = nc.dram_tensor("wk", [D, DLOC], BF16, kind="ExternalInput")
    wv = nc.dram_tensor("wv", [D, DLOC], BF16, kind="ExternalInput")
    wo = nc.dram_tensor("wo", [DLOC, D], BF16, kind="ExternalInput")
    cosT = nc.dram_tensor("cosT", [HD, seq], BF16, kind="ExternalInput")
    sinT = nc.dram_tensor("sinT", [HD, seq], BF16, kind="ExternalInput")
    bqk = nc.dram_tensor("bqk", [2, DLOC], F32, kind="ExternalInput")
    out = nc.dram_tensor("out", [seq, D], BF16, kind="ExternalOutput")

    with tile.TileContext(nc) as tc, ExitStack() as ctx:
        consts = ctx.enter_context(tc.tile_pool(name="consts", bufs=1))
        # warm the exp table set before any real activation
        warm = consts.tile([1, 2], F32)
        nc.vector.memset(warm, 0.0)
        nc.scalar.activation(warm, warm, AF.Exp, scale=1.0)

        # weights as [128, chunk, cols]; row d = c*128 + p
        wq_sb = consts.tile([128, NCH, DLOC], BF16)
        nc.sync.dma_start(out=wq_sb, in_=wq.rearrange("(c p) m -> p c m", p=128))
        wk_sb = consts.tile([128, NCH, DLOC], BF16)
        nc.sync.dma_start(out=wk_sb, in_=wk.rearrange("(c p) m -> p c m", p=128))
        wv_sb = consts.tile([128, NCH, DLOC], BF16)
        nc.sync.dma_start(out=wv_sb, in_=wv.rearrange("(c p) m -> p c m", p=128))
        wo_sb = consts.tile([128, 2, D], BF16)
        nc.sync.dma_start(out=wo_sb, in_=wo.rearrange("(c p) m -> p c m", p=128))
        # cos/sign-folded-sin rows duplicated for the two heads of a pair
        cs_sb = consts.tile([128, seq], BF16)
        nc.sync.dma_start(out=cs_sb[0:HD, :], in_=cosT[:])
        nc.sync.dma_start(out=cs_sb[HD:128, :], in_=cosT[:])
        sn_sb = consts.tile([128, seq], BF16)
        nc.sync.dma_start(out=sn_sb[0:HD, :], in_=sinT[:])
        nc.sync.dma_start(out=sn_sb[HD:128, :], in_=sinT[:])
        ones_sb = consts.tile([128, HD], BF16)
        nc.vector.memset(ones_sb, 1.0)
        if add_qk_bias:
            bqk_sb = consts.tile([128, 2, 2], F32)
            nc.sync.dma_start(
                out=bqk_sb, in_=bqk.rearrange("b (h p) -> p b h", p=128)
            )

        # full x resident in SBUF (bf16), stripe-major DMAs so the first
        # K-projection matmuls can start after ~1/4 of the transfer
        x_sb = consts.tile([128, NCH, seq], BF16)
        for nt in range(NT):
            for c in range(NCH):
                nc.sync.dma_start(
                    out=x_sb[:, c, nt * 512 : nt * 512 + 512],
                    in_=xT[c * 128 : (c + 1) * 128, nt * 512 : nt * 512 + 512],
                )

        # long-lived activation tensors
        acts = ctx.enter_context(tc.tile_pool(name="acts", bufs=1))
        qtr = acts.tile([128, 2, seq], BF16)  # RoPE'd Q^T, head pairs
        ktr = acts.tile([128, 2, seq], BF16)
        v_sb = acts.tile([128, KT, HPC, HD + 1], BF16)  # V natural + ones col
        att = acts.tile([128, 2, seq], BF16)  # normalized attn out ^T
        nc.vector.memset(v_sb[:, :, :, HD : HD + 1], 1.0)

        # pools
        ps = ctx.enter_context(tc.tile_pool(name="ps", bufs=1, space="PSUM"))
        rope_pool = ctx.enter_context(tc.tile_pool(name="rope", bufs=3))
        pt_pool = ctx.enter_context(tc.tile_pool(name="ptp", bufs=6))
        npool = ctx.enter_context(tc.tile_pool(name="norm", bufs=2))
        opool = ctx.enter_context(tc.tile_pool(name="ostage", bufs=2))

        def emit_qk(dst, w_sb, hp, nt, name):
            """Project one [128 dims, 512 tok] stripe of q^T or k^T and RoPE it."""
            cs = slice(nt * 512, nt * 512 + 512)
            pst = ps.tile([128, 512], F32, tag="qkv", bufs=2, name=f"ps_{name}")
            for c in range(NCH):
                nc.tensor.matmul(
                    pst,
                    w_sb[:, c, hp * 128 : hp * 128 + 128],
                    x_sb[:, c, cs],
                    start=(c == 0),
                    stop=(c == NCH - 1),
                )
            raw = rope_pool.tile([128, 512], BF16, tag="raw", name=f"raw_{name}")
            nc.vector.tensor_copy(raw, pst)
            if add_qk_bias:
                bi = 0 if dst is qtr else 1
                nc.vector.tensor_scalar_add(raw, raw, bqk_sb[:, bi, hp : hp + 1])
            rot = rope_pool.tile([128, 512], BF16, tag="rot", name=f"rot_{name}")
            for b in (0, 64):
                nc.vector.tensor_copy(rot[b : b + 32, :], raw[b + 32 : b + 64, :])
                nc.vector.tensor_copy(rot[b + 32 : b + 64, :], raw[b : b + 32, :])
            nc.vector.tensor_mul(rot, rot, sn_sb[:, cs])
            tmp = rope_pool.tile([128, 512], BF16, tag="tmp", name=f"tmp_{name}")
            nc.vector.tensor_mul(tmp, raw, cs_sb[:, cs])
            nc.vector.tensor_add(dst[:, hp, cs], tmp, rot)

        def emit_v_chunk(kt_idx):
            """V natural [128 tokens, vcol] for one k-tile, all 4 heads."""
            vp = ps.tile([128, HPC, HD], F32, tag="qkv", bufs=2, name="ps_v")
            for c in range(NCH):
                nc.tensor.matmul(
                    vp,
                    x_sb[:, c, kt_idx * 128 : kt_idx * 128 + 128],
                    wv_sb[:, c, :],
                    start=(c == 0),
                    stop=(c == NCH - 1),
                )
            nc.vector.tensor_copy(v_sb[:, kt_idx, :, 0:HD], vp)

        def emit_v(nt):
            for tt in range(4):
                emit_v_chunk(nt * 4 + tt)

        def emit_attn(qt, hp, pre_kt=None, pre_pv=None):
            """scores -> exp -> PV -> normalize(A) for one (q-stripe, pair).

            Tile trace order IS program order, so hooks place prerequisite /
            filler work at exact trace positions:
              pre_kt[kt]: before that kt's score matmuls
              pre_pv[kt]: between the exp and that kt's PV matmuls (used for
                          the V chunk feeding exactly that PV)
            Returns a finish closure (norm part B: 1/Z broadcast + multiply)
            that the caller emits later — off the PE-critical path, after the
            reciprocal has had time to complete."""
            qs = slice(qt * 512, qt * 512 + 512)
            pv = [
                ps.tile([128, 512], F32, tag="pv", bufs=2, name="pv0"),
                ps.tile([128, 512], F32, tag="pv", bufs=2, name="pv1"),
            ]
            for kt in range(KT):
                if pre_kt and kt in pre_kt:
                    pre_kt[kt]()
                sc = ps.tile([128, 2, 512], F32, tag="sc", bufs=2, name="sc")
                for h in range(2):
                    hb = h * 64
                    nc.tensor.matmul(
                        sc[:, h, :],
                        ktr[hb : hb + 64, hp, kt * 128 : kt * 128 + 128],
                        qtr[hb : hb + 64, hp, qs],
                        start=True,
                        stop=True,
                    )
                pt = pt_pool.tile([128, 2, 512], BF16, tag="pt", bufs=6, name="pt")
                nc.scalar.activation(pt, sc, AF.Exp, scale=0.125)
                if pre_pv and kt in pre_pv:
                    pre_pv[kt]()
                for h in range(2):
                    nc.tensor.matmul(
                        pv[h][0 : HD + 1, :],
                        v_sb[:, kt, hp * 2 + h, :],
                        pt[:, h, :],
                        start=(kt == 0),
                        stop=(kt == KT - 1),
                        skip_group_check=True,
                    )
            # norm part A: stage O'/Z off PSUM (frees the pv slots) and start
            # the reciprocal. tensor_tensor needs same start partitions, so
            # partition shifts are done with copies (exempt from the rule).
            o_sb = npool.tile([128, 512], BF16, tag="osb", name="osb")
            zstage = npool.tile([65, 512], F32, tag="zstage", name="zstage")
            zrecf = npool.tile([65, 512], F32, tag="zrecf", name="zrecf")
            zrec = npool.tile([65, 512], BF16, tag="zrec", name="zrec")
            for h in range(2):
                hb = h * 64
                nc.vector.tensor_copy(o_sb[hb : hb + 64, :], pv[h][0:HD, :])
                nc.vector.tensor_copy(zstage[hb : hb + 1, :], pv[h][HD : HD + 1, :])
            # one partition-parallel reciprocal covers both heads' Z rows
            # (rows 1..63 are don't-care lanes; same wall time as [1,512])
            nc.vector.reciprocal(zrecf[0:HD + 1, :], zstage[0:HD + 1, :])
            for h in range(2):
                hb = h * 64
                nc.vector.tensor_copy(zrec[hb : hb + 1, :], zrecf[hb : hb + 1, :])

            def finish():
                # norm part B: rank-1 broadcast of 1/Z + un-normalized O' mul
                zb = ps.tile([128, 512], F32, tag="qkv", bufs=2, name="zb")
                for h in range(2):
                    hb = h * 64
                    nc.tensor.matmul(
                        zb[hb : hb + 64, :],
                        ones_sb[hb : hb + 1, 0:HD],
                        zrec[hb : hb + 1, :],
                        start=True,
                        stop=True,
                        skip_group_check=True,
                    )
                    nc.vector.tensor_mul(
                        att[hb : hb + 64, hp, qs],
                        o_sb[hb : hb + 64, :],
                        zb[hb : hb + 64, :],
                    )

            return finish

        def emit_wo(qt):
            """Output projection + store for one 512-token stripe."""
            for tt in range(4):
                tok = qt * 512 + tt * 128
                for nh in range(2):
                    pw = ps.tile([128, 512], F32, tag="qkv", bufs=2, name="ps_wo")
                    for hp in range(2):
                        nc.tensor.matmul(
                            pw,
                            att[:, hp, tok : tok + 128],
                            wo_sb[:, hp, nh * 512 : nh * 512 + 512],
                            start=(hp == 0),
                            stop=(hp == 1),
                        )
                    o_out = opool.tile([128, 512], BF16, tag="oo", name="oo")
                    nc.vector.tensor_copy(o_out, pw)
                    nc.sync.dma_start(
                        out=out[tok : tok + 128, nh * 512 : nh * 512 + 512],
                        in_=o_out,
                    )

        if order == "serial":
            for hp in range(2):
                for nt in range(NT):
                    emit_qk(ktr, wk_sb, hp, nt, f"k{hp}_{nt}")
                    emit_qk(qtr, wq_sb, hp, nt, f"q{hp}_{nt}")
            for nt in range(NT):
                emit_v(nt)
            for hp in range(2):
                for qt in range(NT):
                    emit_attn(qt, hp)()
            for qt in range(NT):
                emit_wo(qt)
        else:
            # Trace order = program order = scheduler priority. Attention
            # feeds the bottleneck ScalarE exp stream, so it leads; K/Q
            # stripe projections, V chunks, deferred norm finishes and WO are
            # threaded into exact positions where their outputs are first
            # needed (or as PE filler).
            emit_qk(ktr, wk_sb, 0, 0, "k0_0")
            emit_qk(qtr, wq_sb, 0, 0, "q0_0")
            fin = emit_attn(
                0, 0,
                pre_kt={4: lambda: emit_qk(ktr, wk_sb, 0, 1, "k0_1"),
                        8: lambda: emit_qk(ktr, wk_sb, 0, 2, "k0_2"),
                        12: lambda: emit_qk(ktr, wk_sb, 0, 3, "k0_3")},
                pre_pv={kt: (lambda k=kt: emit_v_chunk(k)) for kt in range(KT)},
            )
            emit_qk(qtr, wq_sb, 0, 1, "q0_1")
            fin = [fin, emit_attn(
                1, 0,
                pre_kt={0: fin,
                        5: lambda: emit_qk(qtr, wq_sb, 0, 2, "q0_2"),
                        10: lambda: emit_qk(qtr, wq_sb, 0, 3, "q0_3")},
            )][1]
            fin = [fin, emit_attn(
                2, 0,
                pre_kt={0: fin,
                        4: lambda: emit_qk(ktr, wk_sb, 1, 0, "k1_0"),
                        8: lambda: emit_qk(ktr, wk_sb, 1, 1, "k1_1"),
                        12: lambda: emit_qk(ktr, wk_sb, 1, 2, "k1_2")},
            )][1]
            fin = [fin, emit_attn(
                3, 0,
                pre_kt={0: fin,
                        4: lambda: emit_qk(ktr, wk_sb, 1, 3, "k1_3"),
                        8: lambda: emit_qk(qtr, wq_sb, 1, 0, "q1_0"),
                        12: lambda: emit_qk(qtr, wq_sb, 1, 1, "q1_1")},
            )][1]
            fin = [fin, emit_attn(
                0, 1,
                pre_kt={0: fin,
                        5: lambda: emit_qk(qtr, wq_sb, 1, 2, "q1_2"),
                        10: lambda: emit_qk(qtr, wq_sb, 1, 3, "q1_3")},
            )][1]
            fin = [fin, emit_attn(
                1, 1,
                pre_kt={0: fin, 6: lambda: emit_wo(0)},
            )][1]
            fin = [fin, emit_attn(
                2, 1,
                pre_kt={0: fin, 6: lambda: emit_wo(1)},
            )][1]
            fin = [fin, emit_attn(
                3, 1,
                pre_kt={0: fin, 6: lambda: emit_wo(2)},
            )][1]
            fin()
            emit_wo(3)

    _split_sync_waits(nc, maxw=1)
    return nc


_NC_CACHE = {}


def _rope_cos_sin(seq):
    inv_freq = 1.0 / (
        ROPE_BASE ** (np.arange(0, HD, 2, dtype=np.float32) / HD)
    )
    pos = np.arange(seq, dtype=np.float32)
    freqs = pos[:, None] * inv_freq[None, :]  # [seq, 32]
    emb = np.concatenate([freqs, freqs], axis=-1)  # [seq, 64]
    return np.cos(emb).astype(np.float32), np.sin(emb).astype(np.float32)


def build_in_maps(hs, W_qkv, b_qkv, W_o, b_o):
    """Per-core input dict list (shared by kernel() and test harness)."""
    seq = hs.shape[1]
    cos, sin = _rope_cos_sin(seq)
    # sign-folded sin: rows 0..31 negated (multiplies the shifted-up half)
    sin_hat = sin.copy()
    sin_hat[:, :32] *= -1.0
    cosT = np.ascontiguousarray(cos.T).astype(NPBF16)
    sinT = np.ascontiguousarray(sin_hat.T).astype(NPBF16)

    bq, bk = b_qkv[:D], b_qkv[D : 2 * D]
    in_maps = []
    for core in range(N_CORES):
        bb, g = core // 4, core % 4
        cols = slice(g * DLOC, (g + 1) * DLOC)
        in_maps.append(
            {
                "xT": np.ascontiguousarray(hs[bb].T).astype(NPBF16),
                "wq": np.ascontiguousarray(W_qkv[:, :D][:, cols]).astype(NPBF16),
                "wk": np.ascontiguousarray(
                    W_qkv[:, D : 2 * D][:, cols]
                ).astype(NPBF16),
                "wv": np.ascontiguousarray(W_qkv[:, 2 * D :][:, cols]).astype(
                    NPBF16
                ),
                "wo": np.ascontiguousarray(W_o[cols, :]).astype(NPBF16),
                "cosT": cosT,
                "sinT": sinT,
                "bqk": np.stack([bq[cols], bk[cols]]).astype(np.float32),
            }
        )
    return in_maps


def kernel(hidden_states, W_qkv, b_qkv, W_o, b_o):
    hs = np.asarray(hidden_states, dtype=np.float32)
    W_qkv = np.asarray(W_qkv, dtype=np.float32)
    b_qkv = np.asarray(b_qkv, dtype=np.float32)
    W_o = np.asarray(W_o, dtype=np.float32)
    b_o = np.asarray(b_o, dtype=np.float32)
    b, seq, d = hs.shape

    bq, bk, bv = b_qkv[:D], b_qkv[D : 2 * D], b_qkv[2 * D :]
    add_qk_bias = bool(np.any(bq) or np.any(bk))

    key = (seq, add_qk_bias)
    if key not in _NC_CACHE:
        _NC_CACHE[key] = build_attention_nc(seq, add_qk_bias)
    nc = _NC_CACHE[key]

    in_maps = build_in_maps(hs, W_qkv, b_qkv, W_o, b_o)
    res = run_bass_kernel_spmd(nc, in_maps, list(range(N_CORES)))
    parts = [res.results[c]["out"].astype(np.float32) for c in range(N_CORES)]
    outv = np.stack(
        [parts[0] + parts[1] + parts[2] + parts[3],
         parts[4] + parts[5] + parts[6] + parts[7]]
    )
    outv += b_o[None, None, :] + (bv @ W_o)[None, None, :]
    return outv.astype(np.float32)
